# revision 17
# baseline (speedup 1.0000x reference)
"""Trainium2 Bass kernel for nn_F3Layer (gnn_message_passing), 8-core SPMD.

Row-shard n=4096 across 8 cores (NLOC=512 rows/core). Everything on-chip is in
transposed [d, n] layout so the attention softmax normalization and the PV
contraction both contract over the partition axis via matmuls:

  hops:    H1T_loc = sum_kc H_nat[kc]^T @ AT_c[kc]  -> AllGather -> H1T full;
           H2T likewise with PE-transposed H1 as the stationary operand.
  attn:    S_k = (U_k Minv_k)^T @ hopsT;  scoresT[j,m] = S_k[:,jc]^T @ Zloc_k;
           exp on ACT (no row-max: scores are O(1), mask is multiplicative);
           bf16 mask multiply; PV+rowsum in one matmul via a ones-augmented
           ZkT block (33 columns).
  Minv:    6-step Newton (X <- 2X - XMX) from X0 = I/||M||_inf;
  logdet:  15-term trace power series of B = I - M/s; weights = softmax.
           All per-k 32x32 work stays on partition band [32k, 32k+32) - engines
           cannot move data across partitions, so bands never mix except via
           matmuls (which place output by PSUM tile position).
  lap:     L @ (sum_k c_k hops_k) with c_k*I scaled-identity PE transposes.
  tail:    shrink + LayerNorm in T layout (threshold/gamma/beta per-partition).
"""

import numpy as np

import concourse.bass as bass
import concourse.bacc as bacc
import concourse.tile as tile
from concourse import mybir
from concourse.bass_utils import run_bass_kernel_spmd

F32 = mybir.dt.float32
F32R = mybir.dt.float32r
BF16 = mybir.dt.bfloat16
ALU = mybir.AluOpType
ACT = mybir.ActivationFunctionType

ETA, EPS = 0.5, 0.5
N, D, SUB, K = 4096, 128, 32, 3
NC_CORES = 8
NLOC = N // NC_CORES          # 512
KC = N // 128                 # 32 contraction chunks
JC = N // 128                 # 32 j-chunks of scoresT
NEWTON_EXTRA = 5              # after the analytic first step
LOGDET_TERMS = 15
COEFF = SUB / (N * EPS ** 2)
AUG = SUB + 1                 # ZkT columns + ones column


def build_nc():
    nc = bacc.Bacc(None, target_bir_lowering=False, num_devices=NC_CORES)

    h_nat = nc.dram_tensor("h_nat", [N, D], F32R, kind="ExternalInput")
    hT = nc.dram_tensor("hT", [D, N], F32R, kind="ExternalInput")
    hT_loc = nc.dram_tensor("hT_loc", [D, NLOC], F32, kind="ExternalInput")
    at = nc.dram_tensor("at", [N, NLOC], F32R, kind="ExternalInput")
    lt = nc.dram_tensor("lt", [N, NLOC], F32R, kind="ExternalInput")
    maskT = nc.dram_tensor("maskT", [N, NLOC], BF16, kind="ExternalInput")
    u_in = nc.dram_tensor("u", [K, D, SUB], F32R, kind="ExternalInput")
    ut_in = nc.dram_tensor("ut", [K, SUB, D], F32, kind="ExternalInput")
    ident = nc.dram_tensor("ident", [D, D], F32, kind="ExternalInput")
    pcol = nc.dram_tensor("pcol", [D, 8], F32, kind="ExternalInput")
    out_t = nc.dram_tensor("out_t", [D, NLOC], F32, kind="ExternalOutput")
    w_out = nc.dram_tensor("w_out", [1, 4], F32, kind="ExternalOutput")

    with tile.TileContext(nc) as tc:
        with (
            tc.tile_pool(name="res", bufs=1) as res,
            tc.tile_pool(name="stream", bufs=2) as stream,
            tc.tile_pool(name="work", bufs=3) as work,
            tc.tile_pool(name="dram", bufs=1, space="DRAM") as dram,
            tc.tile_pool(name="ps_chain", bufs=2, space="PSUM") as ps_chain,
            tc.tile_pool(name="ps_tr", bufs=1, space="PSUM") as ps_tr,
            tc.tile_pool(name="ps_sc", bufs=2, space="PSUM") as ps_sc,
            tc.tile_pool(name="ps_pv", bufs=1, space="PSUM") as ps_pv,
            tc.tile_pool(name="ps_sm", bufs=2, space="PSUM") as ps_sm,
        ):
            # ================= resident loads =================
            hn = res.tile([128, KC * D], F32R, tag="hopnat", bufs=3)
            nc.sync.dma_start(
                out=hn[:, :].rearrange("p (c d) -> p c d", d=D),
                in_=h_nat[:, :].rearrange("(c p) d -> p c d", p=128),
            )
            h0T = res.tile([128, N], F32R, tag="h0T")
            nc.sync.dma_start(out=h0T[:, :], in_=hT[:, :])
            hTloc_sb = res.tile([128, NLOC], F32, tag="hTloc")
            nc.sync.dma_start(out=hTloc_sb[:, :], in_=hT_loc[:, :])
            hTloc_r = res.tile([128, NLOC], F32R, tag="hTloc_r")
            nc.gpsimd.dma_start(out=hTloc_r[:, :], in_=hT_loc[:, :])
            mask_sb = res.tile([128, KC * NLOC], BF16, tag="mask")
            nc.sync.dma_start(
                out=mask_sb[:, :].rearrange("p (c m) -> p c m", m=NLOC),
                in_=maskT[:, :].rearrange("(c p) m -> p c m", p=128),
            )
            u_sb = res.tile([128, K * SUB], F32R, tag="u")
            nc.sync.dma_start(
                out=u_sb[:, :].rearrange("d (k s) -> d k s", s=SUB),
                in_=u_in[:, :, :].rearrange("k d s -> d k s"),
            )
            ut_sb = res.tile([SUB, K * D], F32, tag="ut")
            nc.sync.dma_start(
                out=ut_sb[:, :].rearrange("s (k d) -> s k d", d=D),
                in_=ut_in[:, :, :].rearrange("k s d -> s k d"),
            )
            id_sb = res.tile([128, D], F32R, tag="ident")
            nc.gpsimd.dma_start(out=id_sb[:, :], in_=ident[:, :])
            id3 = res.tile([K * SUB, SUB], F32, tag="id3")
            for k in range(K):
                nc.sync.dma_start(out=id3[k * SUB:(k + 1) * SUB, :],
                                  in_=ident[0:SUB, 0:SUB])
            pcol_sb = res.tile([128, 8], F32, tag="pcol")
            nc.sync.dma_start(out=pcol_sb[:, :], in_=pcol[:, :])

            ones_col = res.tile([128, 1], F32, tag="ones_col")
            nc.vector.memset(ones_col[:, :], 1.0)
            ones_row = res.tile([1, 128], F32, tag="ones_row")
            nc.vector.memset(ones_row[:, :], 1.0)
            onesk3 = res.tile([K * SUB, SUB], F32, tag="onesk3")
            nc.vector.memset(onesk3[:, :], 1.0)
            nthr = res.tile([128, 1], F32, tag="nthr")
            nc.vector.tensor_scalar_mul(nthr[:, :], pcol_sb[:, 0:1], -1.0)

            # zt_all: per (k, jc) a [128, AUG] bf16 block; col SUB stays 1.0
            zt_all = res.tile([128, K * JC * AUG], BF16, tag="zt")
            nc.vector.memset(zt_all[:, :], 1.0)

            # per-k small matrices, packed on partition bands [32k, 32k+32)
            m_all = res.tile([K * SUB, SUB], F32, tag="m_all")
            b_all = res.tile([K * SUB, SUB], F32, tag="b_all")
            x_all = res.tile([K * SUB, SUB], F32, tag="x_all")
            p_all = res.tile([K * SUB, SUB], F32, tag="p_all")
            vs_all = res.tile([K * SUB, SUB], F32, tag="vs_all")
            t1_sb = res.tile([K * SUB, SUB], F32, tag="t1_sb")
            dg_sb = res.tile([K * SUB, SUB], F32, tag="dg_sb")
            fold_a = res.tile([K * SUB, SUB], F32, tag="fold_a")
            fold_b = res.tile([K * SUB, SUB], F32, tag="fold_b")
            rs_col = res.tile([K * SUB, 1], F32, tag="rs_col")
            c0b = res.tile([K * SUB, 1], F32, tag="c0b")
            nc0b = res.tile([K * SUB, 1], F32, tag="nc0b")
            # per-k scalars on each band's first partition:
            # col 0 = s (inf-norm), 1 = 1/s, 2 = ln s, 3 = tr(V)
            scal = res.tile([K * SUB, 8], F32, tag="scal")
            nc.vector.memset(scal[:, :], 0.0)
            crs3 = res.tile([K * SUB, K], F32, tag="crs3")
            nc.vector.memset(crs3[:, :], 0.0)
            minv0 = res.tile([SUB, K * SUB], F32, tag="minv0")
            w_um = res.tile([128, K * SUB], F32R, tag="w_um")
            zloc_all = res.tile([SUB, K * NLOC], F32R, tag="zloc")
            w_sb = res.tile([1, K], F32, tag="w_sb")
            wb128 = res.tile([128, K], F32, tag="wb128")

            hkT = [h0T, None, None]
            hkTloc = [hTloc_r, None, None]
            gsb = [res.tile([128, NLOC], F32, tag=f"gsb{k}", name=f"gsb{k}")
                   for k in range(K)]
            lap_sb = res.tile([128, NLOC], F32, tag="lap")

            cc_in = [dram.tile([128, NLOC], F32, tag=f"cc_in{i}", name=f"cc_in{i}")
                     for i in range(2)]
            cc_out = [
                dram.tile([128 * NC_CORES, NLOC], F32, addr_space="Shared",
                          tag=f"cc_out{i}", name=f"cc_out{i}")
                for i in range(2)
            ]

            # ---------------- helpers ----------------
            def band_fold(src_col, k, dst_scal_col, op):
                """Reduce a [32,1] band column across partitions, result to
                scal[32k, dst_scal_col]. Stays entirely inside band k via a
                32x32 DVE block transpose."""
                sl = slice(k * SUB, (k + 1) * SUB)
                r0 = slice(k * SUB, k * SUB + 1)
                nc.vector.memset(fold_a[sl, :], 0.0)
                nc.vector.tensor_copy(fold_a[sl, 0:1], src_col)
                nc.vector.transpose(fold_b[sl, :], fold_a[sl, :])
                nc.vector.tensor_reduce(
                    scal[r0, dst_scal_col:dst_scal_col + 1], fold_b[r0, :],
                    axis=mybir.AxisListType.X, op=op,
                )

            def hop_chain(lhs_tiles_ap, dst_name):
                ps = ps_chain.tile([128, NLOC], F32, tag="chain")
                for kc in range(KC):
                    a_t = stream.tile([128, NLOC], F32R, tag="at", bufs=2)
                    nc.sync.dma_start(out=a_t[:, :],
                                      in_=at[kc * 128:(kc + 1) * 128, :])
                    nc.tensor.matmul(
                        ps[:, :], lhs_tiles_ap(kc), a_t[:, :],
                        start=(kc == 0), stop=(kc == KC - 1),
                    )
                loc = res.tile([128, NLOC], F32, tag=dst_name)
                nc.any.tensor_copy(loc[:, :], ps[:, :])
                loc_r = res.tile([128, NLOC], F32R, tag=dst_name + "_r")
                nc.any.tensor_copy(loc_r[:, :], ps[:, :])
                return loc, loc_r

            def all_gather(loc, idx, dst_tag):
                nc.gpsimd.dma_start(out=cc_in[idx][:, :], in_=loc[:, :])
                nc.gpsimd.collective_compute(
                    "AllGather", ALU.bypass,
                    replica_groups=[list(range(NC_CORES))],
                    ins=[cc_in[idx][:, :].opt()],
                    outs=[cc_out[idx][:, :].opt()],
                )
                full = res.tile([128, N], F32R, tag=dst_tag)
                nc.gpsimd.dma_start(
                    out=full[:, :].rearrange("p (r m) -> p r m", m=NLOC),
                    in_=cc_out[idx][:, :].rearrange("(r p) m -> p r m", p=128),
                )
                return full

            def transpose_batch(src_full, dst_tile):
                for g in range(KC // 4):
                    ps = ps_tr.tile([128, 512], F32R, tag="tr")
                    for t in range(4):
                        kc = g * 4 + t
                        nc.tensor.matmul(
                            ps[:, t * 128:(t + 1) * 128],
                            src_full[:, kc * 128:(kc + 1) * 128],
                            id_sb[:, :],
                            is_transpose=True, start=(t == 0), stop=(t == 3),
                        )
                    nc.any.tensor_copy(dst_tile[:, g * 512:(g + 1) * 512],
                                       ps[:, :])

            # ---------------- per-hop small-matrix phase ----------------
            def smalls(k):
                kT = hkT[k]
                kb = k * SUB
                sl = slice(kb, kb + SUB)
                r0 = slice(kb, kb + 1)
                # ZkT tiles, psum-batched 16 per bank
                for g in range(JC // 16):
                    ps = ps_tr.tile([128, 512], F32, tag="tr")
                    for t in range(16):
                        jc = g * 16 + t
                        nc.tensor.matmul(
                            ps[:, t * SUB:(t + 1) * SUB],
                            kT[:, jc * 128:(jc + 1) * 128],
                            u_sb[:, kb:kb + SUB],
                            start=(t == 0), stop=(t == 15),
                        )
                    dst = zt_all[:, (k * JC + g * 16) * AUG:
                                 (k * JC + (g + 1) * 16) * AUG]
                    nc.any.tensor_copy(
                        dst.rearrange("p (t c) -> p t c", c=AUG)[:, :, 0:SUB],
                        ps[:, :].rearrange("p (t c) -> p t c", c=SUB),
                    )
                # Gram
                g_ps = ps_sm.tile([K * SUB, 512], F32, tag="sm")
                for jc in range(JC):
                    base = (k * JC + jc) * AUG
                    zt_jc = zt_all[:, base:base + SUB]
                    nc.tensor.matmul(g_ps[sl, 0:SUB], zt_jc, zt_jc,
                                     start=(jc == 0), stop=(jc == JC - 1))
                nc.vector.scalar_tensor_tensor(
                    m_all[sl, :], g_ps[sl, 0:SUB], float(COEFF), id3[sl, :],
                    op0=ALU.mult, op1=ALU.add,
                )
                # s = ||M||_inf, c0 = 1/s (band scalars)
                nc.vector.tensor_reduce(
                    rs_col[sl, :], m_all[sl, :], axis=mybir.AxisListType.X,
                    op=ALU.add, apply_absolute_value=True,
                )
                band_fold(rs_col[sl, :], k, 0, ALU.max)
                nc.vector.reciprocal(scal[r0, 1:2], scal[r0, 0:1])
                nc.scalar.activation(scal[r0, 2:3], scal[r0, 0:1], ACT.Ln,
                                     bias=scal[r0, 4:5])
                bc = ps_sm.tile([K * SUB, 512], F32, tag="sm")
                nc.tensor.matmul(bc[sl, 0:1], onesk3[r0, :], scal[r0, 1:2],
                                 start=True, stop=True)
                nc.any.tensor_copy(c0b[sl, :], bc[sl, 0:1])
                nc.vector.tensor_scalar_mul(nc0b[sl, :], c0b[sl, :], -1.0)
                # B = I - c0 M ; X1 = c0 (I + B)
                nc.vector.scalar_tensor_tensor(
                    b_all[sl, :], m_all[sl, :], nc0b[sl, 0:1], id3[sl, :],
                    op0=ALU.mult, op1=ALU.add,
                )
                nc.vector.tensor_add(x_all[sl, :], b_all[sl, :], id3[sl, :])
                nc.vector.tensor_scalar_mul(x_all[sl, :], x_all[sl, :],
                                            c0b[sl, 0:1])
                # Newton
                for _ in range(NEWTON_EXTRA):
                    t1 = ps_sm.tile([K * SUB, 512], F32, tag="sm")
                    nc.tensor.matmul(t1[sl, 0:SUB], m_all[sl, :],
                                     x_all[sl, :], start=True, stop=True)
                    nc.any.tensor_copy(t1_sb[sl, :], t1[sl, 0:SUB])
                    t2 = ps_sm.tile([K * SUB, 512], F32, tag="sm")
                    nc.tensor.matmul(t2[sl, 0:SUB], x_all[sl, :],
                                     t1_sb[sl, :], start=True, stop=True)
                    nc.vector.scalar_tensor_tensor(
                        x_all[sl, :], x_all[sl, :], 2.0, t2[sl, 0:SUB],
                        op0=ALU.mult, op1=ALU.subtract,
                    )
                # logdet series
                nc.vector.tensor_copy(p_all[sl, :], b_all[sl, :])
                nc.vector.tensor_copy(vs_all[sl, :], b_all[sl, :])
                for j in range(2, LOGDET_TERMS + 1):
                    pp = ps_sm.tile([K * SUB, 512], F32, tag="sm")
                    nc.tensor.matmul(pp[sl, 0:SUB], b_all[sl, :],
                                     p_all[sl, :], start=True, stop=True)
                    nc.any.tensor_copy(p_all[sl, :], pp[sl, 0:SUB])
                    nc.vector.scalar_tensor_tensor(
                        vs_all[sl, :], p_all[sl, :], 1.0 / j, vs_all[sl, :],
                        op0=ALU.mult, op1=ALU.add,
                    )
                nc.vector.tensor_mul(dg_sb[sl, :], vs_all[sl, :], id3[sl, :])
                nc.vector.tensor_reduce(
                    rs_col[sl, :], dg_sb[sl, :], axis=mybir.AxisListType.X,
                    op=ALU.add,
                )
                band_fold(rs_col[sl, :], k, 3, ALU.add)
                # crs_k = 0.5*SUB*ln(s) - 0.5*tr(V), into crs3[band row, k]
                nc.vector.tensor_scalar_mul(scal[r0, 2:3], scal[r0, 2:3],
                                            0.5 * SUB)
                nc.vector.scalar_tensor_tensor(
                    crs3[r0, k:k + 1], scal[r0, 3:4], -0.5, scal[r0, 2:3],
                    op0=ALU.mult, op1=ALU.add,
                )
                # Minv (= x_all) to base partition 0 via PE transpose (symmetric)
                m0 = ps_sm.tile([SUB, 512], F32, tag="sm")
                nc.tensor.matmul(m0[:, 0:SUB], x_all[sl, :], id3[sl, :],
                                 is_transpose=True, start=True, stop=True)
                nc.any.tensor_copy(minv0[:, kb:kb + SUB], m0[:, 0:SUB])
                # W_um = U_k @ Minv_k  [128, 32]
                wu_ps = ps_sm.tile([D, 512], F32, tag="sm")
                nc.tensor.matmul(wu_ps[:, 0:SUB], ut_sb[:, k * D:(k + 1) * D],
                                 minv0[:, kb:kb + SUB], start=True, stop=True)
                nc.any.tensor_copy(w_um[:, kb:kb + SUB], wu_ps[:, 0:SUB])
                # Zloc_k = U_k^T @ hkT_loc  [32, 512]
                zl_ps = ps_sm.tile([SUB, NLOC], F32, tag="sm")
                nc.tensor.matmul(zl_ps[:, :], u_sb[:, kb:kb + SUB],
                                 hkTloc[k][:, :], start=True, stop=True)
                nc.any.tensor_copy(zloc_all[:, k * NLOC:(k + 1) * NLOC],
                                   zl_ps[:, :])

            # ---------------- per-hop scores / PV phase ----------------
            def scores_pv(k):
                kT = hkT[k]
                kb = k * SUB
                pv_ps = ps_pv.tile([AUG, NLOC], F32, tag="pv")
                for ch in range(8):
                    s_ps = ps_sm.tile([SUB, 512], F32, tag="sm")
                    nc.tensor.matmul(
                        s_ps[:, :], w_um[:, kb:kb + SUB],
                        kT[:, ch * 512:(ch + 1) * 512],
                        start=True, stop=True,
                    )
                    s_tmp = work.tile([SUB, 512], F32R, tag="s_tmp", bufs=2)
                    nc.any.tensor_copy(s_tmp[:, :], s_ps[:, :])
                    for t in range(4):
                        jc = ch * 4 + t
                        sc_ps = ps_sc.tile([128, NLOC], F32, tag="sc")
                        nc.tensor.matmul(
                            sc_ps[:, :], s_tmp[:, t * 128:(t + 1) * 128],
                            zloc_all[:, k * NLOC:(k + 1) * NLOC],
                            start=True, stop=True,
                        )
                        texp = work.tile([128, NLOC], BF16, tag="texp", bufs=2)
                        nc.scalar.activation(texp[:, :], sc_ps[:, :], ACT.Exp)
                        expt = work.tile([128, NLOC], BF16, tag="expt", bufs=2)
                        nc.vector.tensor_mul(
                            expt[:, :], texp[:, :],
                            mask_sb[:, jc * NLOC:(jc + 1) * NLOC],
                        )
                        base = (k * JC + jc) * AUG
                        nc.tensor.matmul(
                            pv_ps[:, :], zt_all[:, base:base + AUG], expt[:, :],
                            start=(jc == 0), stop=(jc == JC - 1),
                        )
                # normalize; unweighted gradT into gsb[k]
                pv_sb = work.tile([AUG, NLOC], F32, tag="pv_sb", bufs=1)
                nc.any.tensor_copy(pv_sb[:, :], pv_ps[:, :])
                nc.vector.reciprocal(pv_sb[SUB:AUG, :], pv_sb[SUB:AUG, :])
                rb_ps = ps_sm.tile([SUB, NLOC], F32, tag="sm")
                nc.tensor.matmul(rb_ps[:, :], onesk3[SUB:SUB + 1, :],
                                 pv_sb[SUB:AUG, :], start=True, stop=True)
                rn_sb = work.tile([SUB, NLOC], F32, tag="rn_sb", bufs=1)
                nc.any.tensor_copy(rn_sb[:, :], rb_ps[:, :])
                vkn = work.tile([SUB, NLOC], F32, tag="vkn", bufs=1)
                nc.vector.tensor_mul(vkn[:, :], pv_sb[0:SUB, :], rn_sb[:, :])
                # lhsT for grad: (U_k Minv_k)^T = Minv_k @ U_k^T
                wut_ps = ps_sm.tile([SUB, 512], F32, tag="sm")
                nc.tensor.matmul(wut_ps[:, 0:D], minv0[:, kb:kb + SUB],
                                 ut_sb[:, k * D:(k + 1) * D],
                                 start=True, stop=True)
                wut_sb = work.tile([SUB, D], F32, tag="wut", bufs=1)
                nc.any.tensor_copy(wut_sb[:, :], wut_ps[:, 0:D])
                g_ps = ps_sc.tile([128, NLOC], F32, tag="sc")
                nc.tensor.matmul(g_ps[:, :], wut_sb[:, :], vkn[:, :],
                                 start=True, stop=True)
                nc.any.tensor_copy(gsb[k][:, :], g_ps[:, :])

            # ================= pipeline =================
            h1tloc, h1tloc_r = hop_chain(
                lambda kc: hn[:, kc * D:(kc + 1) * D], "h1tloc")
            hkTloc[1] = h1tloc_r
            h1T = all_gather(h1tloc, 0, "h1T")
            hkT[1] = h1T

            smalls(0)
            scores_pv(0)

            h1n = res.tile([128, KC * D], F32R, tag="hopnat", bufs=3)
            transpose_batch(h1T, h1n)
            h2tloc, h2tloc_r = hop_chain(
                lambda kc: h1n[:, kc * D:(kc + 1) * D], "h2tloc")
            hkTloc[2] = h2tloc_r
            h2T = all_gather(h2tloc, 1, "h2T")
            hkT[2] = h2T

            smalls(1)
            scores_pv(1)

            h2n = res.tile([128, KC * D], F32R, tag="hopnat", bufs=3)
            transpose_batch(h2T, h2n)
            # hn <- c0*hn + c1*h1n + c2*h2n  (becomes Hmix natural, in place)
            nc.vector.tensor_scalar_mul(hn[:, :], hn[:, :], pcol_sb[:, 3:4])
            nc.vector.scalar_tensor_tensor(
                hn[:, :], h1n[:, :], pcol_sb[:, 4:5], hn[:, :],
                op0=ALU.mult, op1=ALU.add,
            )
            nc.vector.scalar_tensor_tensor(
                hn[:, :], h2n[:, :], pcol_sb[:, 5:6], hn[:, :],
                op0=ALU.mult, op1=ALU.add,
            )
            ps_lap = ps_chain.tile([128, NLOC], F32, tag="chain")
            for kc in range(KC):
                l_t = stream.tile([128, NLOC], F32R, tag="lt", bufs=2)
                nc.sync.dma_start(out=l_t[:, :],
                                  in_=lt[kc * 128:(kc + 1) * 128, :])
                nc.tensor.matmul(
                    ps_lap[:, :], hn[:, kc * D:(kc + 1) * D], l_t[:, :],
                    start=(kc == 0), stop=(kc == KC - 1),
                )
            nc.any.tensor_copy(lap_sb[:, :], ps_lap[:, :])

            smalls(2)
            scores_pv(2)

            # ---- weights = softmax(crs) ----
            crs_ps = ps_sm.tile([1, 512], F32, tag="sm")
            nc.tensor.matmul(crs_ps[:, 0:K], ones_col[0:K * SUB, :],
                             crs3[:, :], start=True, stop=True)
            crs_sb = work.tile([1, K], F32, tag="crs_sb", bufs=1)
            nc.any.tensor_copy(crs_sb[:, :], crs_ps[:, 0:K])
            cmax = work.tile([1, 1], F32, tag="cmax", bufs=1)
            nc.vector.tensor_reduce(cmax[:, :], crs_sb[:, :],
                                    axis=mybir.AxisListType.X, op=ALU.max)
            nmax = work.tile([1, 1], F32, tag="nmax", bufs=1)
            nc.vector.tensor_scalar_mul(nmax[:, :], cmax[:, :], -1.0)
            wexp = work.tile([1, K], F32, tag="wexp", bufs=1)
            nc.scalar.activation(wexp[:, :], crs_sb[:, :], ACT.Exp,
                                 bias=nmax[0:1, 0:1])
            wsum = work.tile([1, 1], F32, tag="wsum", bufs=1)
            nc.vector.tensor_reduce(wsum[:, :], wexp[:, :],
                                    axis=mybir.AxisListType.X, op=ALU.add)
            wrec = work.tile([1, 1], F32, tag="wrec", bufs=1)
            nc.vector.reciprocal(wrec[:, :], wsum[:, :])
            nc.vector.tensor_scalar_mul(w_sb[:, :], wexp[:, :], wrec[0:1, 0:1])
            wse = work.tile([1, K], F32, tag="wse", bufs=1)
            nc.vector.tensor_scalar_mul(wse[:, :], w_sb[:, :], float(ETA))
            wb_ps = ps_sm.tile([128, 512], F32, tag="sm")
            nc.tensor.matmul(wb_ps[:, 0:K], ones_row[:, :], wse[:, :],
                             start=True, stop=True)
            nc.any.tensor_copy(wb128[:, :], wb_ps[:, 0:K])
            wpad = work.tile([1, 4], F32, tag="wpad", bufs=1)
            nc.vector.memset(wpad[:, :], 0.0)
            nc.vector.tensor_copy(wpad[:, 0:K], w_sb[:, :])
            nc.sync.dma_start(out=w_out[:, :], in_=wpad[:, :])

            # ================= finale =================
            agg_sb = work.tile([128, NLOC], F32, tag="agg", bufs=1)
            nc.vector.tensor_scalar_mul(agg_sb[:, :], gsb[0][:, :],
                                        wb128[:, 0:1])
            for k in range(1, K):
                nc.vector.scalar_tensor_tensor(
                    agg_sb[:, :], gsb[k][:, :], wb128[:, k:k + 1], agg_sb[:, :],
                    op0=ALU.mult, op1=ALU.add,
                )
            half = work.tile([128, NLOC], F32, tag="fin", bufs=2)
            nc.vector.tensor_add(half[:, :], hTloc_sb[:, :], agg_sb[:, :])
            nc.vector.tensor_sub(half[:, :], half[:, :], lap_sb[:, :])
            pban = work.tile([128, NLOC], F32, tag="fin", bufs=2)
            nc.scalar.activation(pban[:, :], half[:, :], ACT.Relu,
                                 bias=nthr[:, 0:1], scale=1.0)
            nc.scalar.activation(half[:, :], half[:, :], ACT.Relu,
                                 bias=nthr[:, 0:1], scale=-1.0)
            hs = work.tile([128, NLOC], F32, tag="hs", bufs=1)
            nc.vector.tensor_sub(hs[:, :], pban[:, :], half[:, :])
            sum_ps = ps_sm.tile([1, NLOC], F32, tag="sm")
            nc.tensor.matmul(sum_ps[:, :], ones_col[:, :], hs[:, :],
                             start=True, stop=True)
            sq = work.tile([128, NLOC], F32, tag="fin", bufs=2)
            nc.scalar.activation(sq[:, :], hs[:, :], ACT.Square)
            ssq_ps = ps_sm.tile([1, NLOC], F32, tag="sm")
            nc.tensor.matmul(ssq_ps[:, :], ones_col[:, :], sq[:, :],
                             start=True, stop=True)
            mu = work.tile([1, NLOC], F32, tag="mu", bufs=1)
            nc.vector.tensor_scalar_mul(mu[:, :], sum_ps[:, :], 1.0 / D)
            ex2 = work.tile([1, NLOC], F32, tag="ex2", bufs=1)
            nc.vector.tensor_scalar_mul(ex2[:, :], ssq_ps[:, :], 1.0 / D)
            var = work.tile([1, NLOC], F32, tag="var", bufs=1)
            nc.vector.tensor_mul(var[:, :], mu[:, :], mu[:, :])
            nc.vector.tensor_sub(var[:, :], ex2[:, :], var[:, :])
            eps_t = work.tile([1, 1], F32, tag="eps_t", bufs=1)
            nc.vector.memset(eps_t[:, :], 1e-5)
            stdv = work.tile([1, NLOC], F32, tag="stdv", bufs=1)
            nc.scalar.activation(stdv[:, :], var[:, :], ACT.Sqrt,
                                 bias=eps_t[0:1, 0:1])
            rstd = work.tile([1, NLOC], F32, tag="rstd", bufs=1)
            nc.vector.reciprocal(rstd[:, :], stdv[:, :])
            mu_ps = ps_chain.tile([128, NLOC], F32, tag="chain")
            nc.tensor.matmul(mu_ps[:, :], ones_row[:, :], mu[:, :],
                             start=True, stop=True)
            rs_ps = ps_chain.tile([128, NLOC], F32, tag="chain")
            nc.tensor.matmul(rs_ps[:, :], ones_row[:, :], rstd[:, :],
                             start=True, stop=True)
            n1 = work.tile([128, NLOC], F32, tag="fin", bufs=2)
            nc.vector.tensor_sub(n1[:, :], hs[:, :], mu_ps[:, :])
            n2 = work.tile([128, NLOC], F32, tag="fin", bufs=2)
            nc.vector.tensor_mul(n2[:, :], n1[:, :], rs_ps[:, :])
            outsb = work.tile([128, NLOC], F32, tag="fin", bufs=2)
            nc.vector.tensor_scalar(
                outsb[:, :], n2[:, :], pcol_sb[:, 1:2], pcol_sb[:, 2:3],
                op0=ALU.mult, op1=ALU.add,
            )
            nc.sync.dma_start(out=out_t[:, :], in_=outsb[:, :])

    return nc


def _host_prep(H, A, adj_mask, L, U, lambda_laps, threshold, ln_gamma, ln_beta):
    import ml_dtypes
    H = np.ascontiguousarray(np.asarray(H, np.float32))
    A = np.ascontiguousarray(np.asarray(A, np.float32))
    adj_mask = np.asarray(adj_mask, np.float32)
    L = np.ascontiguousarray(np.asarray(L, np.float32))
    U = np.ascontiguousarray(np.asarray(U, np.float32))
    lam = np.asarray(lambda_laps, np.float32)
    laps = np.log1p(np.exp(lam.astype(np.float64))).astype(np.float32)
    cks = (ETA * laps).astype(np.float32)

    HT = np.ascontiguousarray(H.T)
    UT = np.ascontiguousarray(np.transpose(U, (0, 2, 1)))
    eye = np.eye(D, dtype=np.float32)
    pcol = np.zeros((D, 8), np.float32)
    pcol[:, 0] = np.asarray(threshold, np.float32)
    pcol[:, 1] = np.asarray(ln_gamma, np.float32)
    pcol[:, 2] = np.asarray(ln_beta, np.float32)
    pcol[:, 3:6] = cks[None, :]

    in_maps = []
    for c in range(NC_CORES):
        r0, r1 = c * NLOC, (c + 1) * NLOC
        in_maps.append({
            "h_nat": H,
            "hT": HT,
            "hT_loc": np.ascontiguousarray(HT[:, r0:r1]),
            "at": np.ascontiguousarray(A[r0:r1].T),
            "lt": np.ascontiguousarray(L[r0:r1].T),
            "maskT": np.ascontiguousarray(adj_mask[r0:r1].T).astype(
                ml_dtypes.bfloat16),
            "u": U,
            "ut": UT,
            "ident": eye,
            "pcol": pcol,
        })
    return in_maps


_NC_CACHE = None
LAST_EXEC_NS = None
LAST_RESULT = None


def kernel(H, A, adj_mask, L, U, lambda_laps, threshold, ln_gamma, ln_beta):
    global _NC_CACHE
    in_maps = _host_prep(H, A, adj_mask, L, U, lambda_laps, threshold,
                         ln_gamma, ln_beta)
    if _NC_CACHE is None:
        _NC_CACHE = build_nc()
        if not _NC_CACHE.is_finalized():
            _NC_CACHE.finalize()
    nc = _NC_CACHE
    res = run_bass_kernel_spmd(nc, in_maps, core_ids=list(range(NC_CORES)))
    global LAST_EXEC_NS, LAST_RESULT
    LAST_EXEC_NS = res.exec_time_ns
    LAST_RESULT = res
    outs = res.results
    H_out = np.concatenate(
        [np.asarray(outs[c]["out_t"], np.float32).T for c in range(NC_CORES)],
        axis=0,
    )
    weights = np.asarray(outs[0]["w_out"], np.float32)[0, :K].copy()
    return H_out, weights
